# revision 13
# baseline (speedup 1.0000x reference)
"""Trainium2 Bass kernel for nn_ErrorAwareEdgeLoss.

reference:  cost[b,e] = sum_{p,q} P[b,i_e,p] * d_error[p,q] * P[b,j_e,q]
            result    = mean_{b,e} cost[b,e]

The edge pairs only enter through the count matrix
    C[l1,l2] = #edges e with (i_e,j_e) == (l1,l2),
and since d_error is symmetric the result collapses to
    result = <d_error, sum_b Q_b^T Cs Q_b> / (B*E),
with Q_b = P[b,:64,:] and Cs = (C + C^T)/2 (exact in bf16: half-integers).

Device work per core (256 batches, data-parallel over batch):
  - DMA P[b,:64,:] for 32 batches at a time into a [128, 2048] f32 tile
    (two batches stacked on the 128 partitions, 16 batch-pairs wide).
  - cast to bf16 (DVE)
  - Y = Cs @ Q via two concurrent 64x64-quadrant matmuls per 512-wide slab
  - R += QQ^T @ YY (K=128 contraction = 2 batches) accumulated in PSUM f32
  - write the per-core R (128x128 f32) to DRAM.
Host: R_total = sum_c R_c ;  result = <d_error, R_total> / (B*E) in f64.
"""

import sys

_TRN_REPO = "/opt/trn_rl_repo"
if _TRN_REPO not in sys.path:
    sys.path.insert(0, _TRN_REPO)

import numpy as np
import ml_dtypes

B, L, H = 2048, 64, 128     # batch, logical qubits, physical dim
E = 512                     # number of circuit edges
N_CORES = 8
BPC = B // N_CORES          # 256 batches per core
GROUP = 32                  # batches per DMA group
NGROUPS = BPC // GROUP      # 8
ROWS_PER_GROUP = GROUP * L  # 2048 DRAM rows per group
FREE = GROUP // 2 * H       # 2048 f32 per partition per group
SLAB = 512                  # matmul moving-operand width
NSLABS = FREE // SLAB       # 4
NBLK = GROUP // 2           # 16 K=128 blocks (2 batches each)

_CACHE = {}

# load variants:
#   "swdge_cast": one gpsimd DMA per group, f32->bf16 cast in the DMA
#   "hwdge_split": sync-engine f32 DMAs split into DMA_SPLIT pieces + DVE cast
LOAD_MODE = "swdge_cast"
DMA_SPLIT = 4
# group sizes in 8-batch slabs (sum must be 32 = 256 batches / 8).
# Small leading groups start the PE early (HWDGE path is up before the
# SWDGE rings are initialized); big trailing groups amortize DMA overhead.
GROUP_SLABS = [1, 1, 2, 4, 8, 8, 8]
HWDGE_HEAD = 2  # this many leading groups load via sync f32 + DVE cast


def _build(load_mode=None):
    import concourse.tile as tile
    from concourse import bacc, mybir

    if load_mode is None:
        load_mode = LOAD_MODE
    f32 = mybir.dt.float32
    bf16 = mybir.dt.bfloat16

    nc = bacc.Bacc(None)
    # host-packed shard: row p holds, concatenated over (group, batch-pair),
    # the 128 floats of Q[g*32 + 2j + p//64, p%64, :] — so every per-group
    # load is a plain 2D DMA with an 8KB contiguous run per partition.
    pq = nc.dram_tensor("pq", [128, NGROUPS * FREE], f32, kind="ExternalInput")
    cs = nc.dram_tensor("cs", [L, L], bf16, kind="ExternalInput")
    r_out = nc.dram_tensor("r_out", [H, H], f32, kind="ExternalOutput")

    assert sum(GROUP_SLABS) * SLAB == NGROUPS * FREE

    with tile.TileContext(nc) as tc:
        with (
            tc.tile_pool(name="singles", bufs=1) as singles,
            tc.tile_pool(name="qraw", bufs=2) as qraw_pool,
            tc.tile_pool(name="qbfp", bufs=4) as qbf_pool,
            tc.tile_pool(name="ybfp", bufs=4) as ybf_pool,
            tc.tile_pool(name="yps", bufs=4, space="PSUM") as yps,
            tc.tile_pool(name="rps", bufs=1, space="PSUM") as rps,
        ):
            # Cs replicated into both partition halves so the two PE
            # quadrants (0,0) and (64,64) each see it as lhsT.
            cs2 = singles.tile([128, L], bf16)
            nc.sync.dma_start(out=cs2[0:64, :], in_=cs[:, :])
            nc.sync.dma_start(out=cs2[64:128, :], in_=cs[:, :])

            r_psum = rps.tile([128, H], f32)

            def load_group(gi, c0, width):
                qbf = qbf_pool.tile([128, width], bf16)
                if load_mode == "swdge_cast" and gi >= HWDGE_HEAD:
                    nc.gpsimd.dma_start(out=qbf[:, :], in_=pq[:, c0 : c0 + width])
                else:
                    qf32 = qraw_pool.tile([128, width], f32)
                    per = width // DMA_SPLIT if load_mode != "swdge_cast" else width
                    for dd in range(width // per):
                        nc.sync.dma_start(
                            out=qf32[:, dd * per : (dd + 1) * per],
                            in_=pq[:, c0 + dd * per : c0 + (dd + 1) * per],
                        )
                    nc.vector.tensor_copy(qbf[:, :], qf32[:, :])
                return qbf

            def emit_y(qbf, nslabs):
                ybf = ybf_pool.tile([128, nslabs * SLAB], bf16)
                for s in range(nslabs):
                    yy = yps.tile([128, SLAB], f32)
                    sl = slice(s * SLAB, (s + 1) * SLAB)
                    nc.tensor.matmul(
                        yy[0:64, :], lhsT=cs2[0:64, :], rhs=qbf[0:64, sl],
                        start=True, stop=True, skip_group_check=True,
                    )
                    nc.tensor.matmul(
                        yy[64:128, :], lhsT=cs2[64:128, :], rhs=qbf[64:128, sl],
                        start=True, stop=True, skip_group_check=True,
                    )
                    # PSUM -> SBUF cast copy, halves on DVE and ACT
                    half = SLAB // 2
                    nc.vector.tensor_copy(
                        ybf[:, s * SLAB : s * SLAB + half], yy[:, 0:half]
                    )
                    nc.scalar.copy(
                        ybf[:, s * SLAB + half : (s + 1) * SLAB],
                        yy[:, half:SLAB],
                    )
                return ybf

            _flags = {"first": True}

            def emit_r(qbf, ybf, nslabs, is_last_group):
                nblocks = nslabs * 4
                for k in range(nblocks):
                    first = _flags["first"]
                    _flags["first"] = False
                    last = is_last_group and k == nblocks - 1
                    nc.tensor.matmul(
                        r_psum[:, :],
                        lhsT=qbf[:, k * H : (k + 1) * H],
                        rhs=ybf[:, k * H : (k + 1) * H],
                        start=first, stop=last, skip_group_check=True,
                    )

            # Software pipeline: R-matmuls run one group behind the
            # Y-matmuls so the PE never waits on the PSUM->SBUF casts.
            prev = None
            c0 = 0
            for gi, k in enumerate(GROUP_SLABS):
                width = k * SLAB
                qbf = load_group(gi, c0, width)
                c0 += width
                ybf = emit_y(qbf, k)
                if prev is not None:
                    emit_r(*prev, is_last_group=False)
                prev = (qbf, ybf, k)
            emit_r(*prev, is_last_group=True)

            rsb = singles.tile([128, H], f32)
            nc.vector.tensor_copy(rsb[:, :], r_psum[:, :])
            nc.sync.dma_start(out=r_out[:, :], in_=rsb[:, :])

    nc.compile()
    return nc


def get_nc():
    key = ("nc", LOAD_MODE, DMA_SPLIT)
    if key not in _CACHE:
        _CACHE[key] = _build()
    return _CACHE[key]


def make_count_matrix(circuit_edge_pairs):
    pairs = np.asarray(circuit_edge_pairs).astype(np.int64)
    C = np.zeros((L, L), np.float64)
    np.add.at(C, (pairs[:, 0], pairs[:, 1]), 1.0)
    Cs = (C + C.T) * 0.5
    return Cs.astype(ml_dtypes.bfloat16)


def pack_shard(Q):
    """(256, 64, 128) f32 -> (128, 16384) so partition p's per-group data
    T[p, g*2048 + j*128 + h] = Q[g*32 + 2j + p//64, p%64, h] is contiguous."""
    arr = Q.reshape(NGROUPS, GROUP // 2, 2, L, H)
    return np.ascontiguousarray(
        arr.transpose(2, 3, 0, 1, 4).reshape(128, NGROUPS * FREE)
    )


def make_in_maps(P, circuit_edge_pairs):
    P = np.asarray(P)
    csb = make_count_matrix(circuit_edge_pairs)
    in_maps = []
    for c in range(N_CORES):
        shard = np.ascontiguousarray(
            P[c * BPC : (c + 1) * BPC, :L, :], dtype=np.float32
        )
        in_maps.append({"pq": pack_shard(shard), "cs": csb})
    return in_maps


def reduce_results(per_core_r, d_error):
    R = np.zeros((H, H), np.float64)
    for r in per_core_r:
        R += np.asarray(r).astype(np.float64)
    out = (np.asarray(d_error).astype(np.float64) * R).sum() / (B * E)
    return np.array(out, dtype=np.float32)


def run_spmd(P, circuit_edge_pairs, **kwargs):
    """Run on the 8 NeuronCores; returns (list of per-core R, BassKernelResults)."""
    from concourse.bass_utils import run_bass_kernel_spmd

    nc = get_nc()
    in_maps = make_in_maps(P, circuit_edge_pairs)
    res = run_bass_kernel_spmd(nc, in_maps, core_ids=list(range(N_CORES)), **kwargs)
    per_core_r = [res.results[c]["r_out"] for c in range(N_CORES)]
    return per_core_r, res


def kernel(P, d_error, circuit_edge_pairs, num_logical):
    assert int(num_logical) == L
    per_core_r, _ = run_spmd(P, circuit_edge_pairs)
    return reduce_results(per_core_r, d_error)


# revision 14
# speedup vs baseline: 1.3067x; 1.3067x over previous
"""Trainium2 Bass kernel for nn_ErrorAwareEdgeLoss.

reference:  cost[b,e] = sum_{p,q} P[b,i_e,p] * d_error[p,q] * P[b,j_e,q]
            result    = mean_{b,e} cost[b,e]

The edge pairs only enter through the count matrix
    C[l1,l2] = #edges e with (i_e,j_e) == (l1,l2),
and since d_error is symmetric the result collapses to
    result = <d_error, sum_b Q_b^T Cs Q_b> / (B*E),
with Q_b = P[b,:64,:] and Cs = (C + C^T)/2 (exact in bf16: half-integers).

Device work per core (256 batches, data-parallel over batch):
  - DMA P[b,:64,:] for 32 batches at a time into a [128, 2048] f32 tile
    (two batches stacked on the 128 partitions, 16 batch-pairs wide).
  - cast to bf16 (DVE)
  - Y = Cs @ Q via two concurrent 64x64-quadrant matmuls per 512-wide slab
  - R += QQ^T @ YY (K=128 contraction = 2 batches) accumulated in PSUM f32
  - write the per-core R (128x128 f32) to DRAM.
Host: R_total = sum_c R_c ;  result = <d_error, R_total> / (B*E) in f64.
"""

import sys

_TRN_REPO = "/opt/trn_rl_repo"
if _TRN_REPO not in sys.path:
    sys.path.insert(0, _TRN_REPO)

import numpy as np
import ml_dtypes

B, L, H = 2048, 64, 128     # batch, logical qubits, physical dim
E = 512                     # number of circuit edges
N_CORES = 8
BPC = B // N_CORES          # 256 batches per core
GROUP = 32                  # batches per DMA group
NGROUPS = BPC // GROUP      # 8
ROWS_PER_GROUP = GROUP * L  # 2048 DRAM rows per group
FREE = GROUP // 2 * H       # 2048 f32 per partition per group
SLAB = 512                  # matmul moving-operand width
NSLABS = FREE // SLAB       # 4
NBLK = GROUP // 2           # 16 K=128 blocks (2 batches each)

_CACHE = {}

# load variants:
#   "swdge_cast": one gpsimd DMA per group, f32->bf16 cast in the DMA
#   "hwdge_split": sync-engine f32 DMAs split into DMA_SPLIT pieces + DVE cast
LOAD_MODE = "swdge_cast"
DMA_SPLIT = 4
# group sizes in 8-batch slabs (sum must be 32 = 256 batches / 8).
# Small leading groups start the PE early (HWDGE path is up before the
# SWDGE rings are initialized); big trailing groups amortize DMA overhead.
GROUP_SLABS = [4, 4, 4, 4, 4, 4, 4, 4]
HWDGE_HEAD = 0  # this many leading groups load via sync f32 + DVE cast


def _build(load_mode=None):
    import concourse.tile as tile
    from concourse import bacc, mybir

    if load_mode is None:
        load_mode = LOAD_MODE
    f32 = mybir.dt.float32
    bf16 = mybir.dt.bfloat16

    nc = bacc.Bacc(None)
    # host-packed shard: row p holds, concatenated over (group, batch-pair),
    # the 128 floats of Q[g*32 + 2j + p//64, p%64, :] — so every per-group
    # load is a plain 2D DMA with an 8KB contiguous run per partition.
    pq = nc.dram_tensor("pq", [128, NGROUPS * FREE], f32, kind="ExternalInput")
    cs = nc.dram_tensor("cs", [L, L], bf16, kind="ExternalInput")
    r_out = nc.dram_tensor("r_out", [H, H], f32, kind="ExternalOutput")

    assert sum(GROUP_SLABS) * SLAB == NGROUPS * FREE

    with tile.TileContext(nc) as tc:
        with (
            tc.tile_pool(name="singles", bufs=1) as singles,
            tc.tile_pool(name="qraw", bufs=2) as qraw_pool,
            tc.tile_pool(name="qbfp", bufs=4) as qbf_pool,
            tc.tile_pool(name="ybfp", bufs=4) as ybf_pool,
            tc.tile_pool(name="yps", bufs=4, space="PSUM") as yps,
            tc.tile_pool(name="rps", bufs=1, space="PSUM") as rps,
        ):
            # Cs replicated into both partition halves so the two PE
            # quadrants (0,0) and (64,64) each see it as lhsT.
            cs2 = singles.tile([128, L], bf16)
            nc.sync.dma_start(out=cs2[0:64, :], in_=cs[:, :])
            nc.sync.dma_start(out=cs2[64:128, :], in_=cs[:, :])

            r_psum = rps.tile([128, H], f32)

            def load_group(gi, c0, width):
                qbf = qbf_pool.tile([128, width], bf16)
                if load_mode == "swdge_cast" and gi >= HWDGE_HEAD:
                    nc.gpsimd.dma_start(out=qbf[:, :], in_=pq[:, c0 : c0 + width])
                else:
                    qf32 = qraw_pool.tile([128, width], f32)
                    per = width // DMA_SPLIT if load_mode != "swdge_cast" else width
                    for dd in range(width // per):
                        nc.sync.dma_start(
                            out=qf32[:, dd * per : (dd + 1) * per],
                            in_=pq[:, c0 + dd * per : c0 + (dd + 1) * per],
                        )
                    nc.vector.tensor_copy(qbf[:, :], qf32[:, :])
                return qbf

            def emit_y(qbf, nslabs):
                ybf = ybf_pool.tile([128, nslabs * SLAB], bf16)
                for s in range(nslabs):
                    yy = yps.tile([128, SLAB], f32)
                    sl = slice(s * SLAB, (s + 1) * SLAB)
                    nc.tensor.matmul(
                        yy[0:64, :], lhsT=cs2[0:64, :], rhs=qbf[0:64, sl],
                        start=True, stop=True, skip_group_check=True,
                    )
                    nc.tensor.matmul(
                        yy[64:128, :], lhsT=cs2[64:128, :], rhs=qbf[64:128, sl],
                        start=True, stop=True, skip_group_check=True,
                    )
                    # PSUM -> SBUF cast copy, halves on DVE and ACT
                    half = SLAB // 2
                    nc.vector.tensor_copy(
                        ybf[:, s * SLAB : s * SLAB + half], yy[:, 0:half]
                    )
                    nc.scalar.copy(
                        ybf[:, s * SLAB + half : (s + 1) * SLAB],
                        yy[:, half:SLAB],
                    )
                return ybf

            _flags = {"first": True}

            def emit_r(qbf, ybf, nslabs, is_last_group):
                nblocks = nslabs * 4
                for k in range(nblocks):
                    first = _flags["first"]
                    _flags["first"] = False
                    last = is_last_group and k == nblocks - 1
                    nc.tensor.matmul(
                        r_psum[:, :],
                        lhsT=qbf[:, k * H : (k + 1) * H],
                        rhs=ybf[:, k * H : (k + 1) * H],
                        start=first, stop=last, skip_group_check=True,
                    )

            # Software pipeline: R-matmuls run one group behind the
            # Y-matmuls so the PE never waits on the PSUM->SBUF casts.
            prev = None
            c0 = 0
            for gi, k in enumerate(GROUP_SLABS):
                width = k * SLAB
                qbf = load_group(gi, c0, width)
                c0 += width
                ybf = emit_y(qbf, k)
                if prev is not None:
                    emit_r(*prev, is_last_group=False)
                prev = (qbf, ybf, k)
            emit_r(*prev, is_last_group=True)

            rsb = singles.tile([128, H], f32)
            nc.vector.tensor_copy(rsb[:, :], r_psum[:, :])
            nc.sync.dma_start(out=r_out[:, :], in_=rsb[:, :])

    nc.compile()
    return nc


def get_nc():
    key = ("nc", LOAD_MODE, DMA_SPLIT)
    if key not in _CACHE:
        _CACHE[key] = _build()
    return _CACHE[key]


def make_count_matrix(circuit_edge_pairs):
    pairs = np.asarray(circuit_edge_pairs).astype(np.int64)
    C = np.zeros((L, L), np.float64)
    np.add.at(C, (pairs[:, 0], pairs[:, 1]), 1.0)
    Cs = (C + C.T) * 0.5
    return Cs.astype(ml_dtypes.bfloat16)


def pack_shard(Q):
    """(256, 64, 128) f32 -> (128, 16384) so partition p's per-group data
    T[p, g*2048 + j*128 + h] = Q[g*32 + 2j + p//64, p%64, h] is contiguous."""
    arr = Q.reshape(NGROUPS, GROUP // 2, 2, L, H)
    return np.ascontiguousarray(
        arr.transpose(2, 3, 0, 1, 4).reshape(128, NGROUPS * FREE)
    )


def make_in_maps(P, circuit_edge_pairs):
    P = np.asarray(P)
    csb = make_count_matrix(circuit_edge_pairs)
    in_maps = []
    for c in range(N_CORES):
        shard = np.ascontiguousarray(
            P[c * BPC : (c + 1) * BPC, :L, :], dtype=np.float32
        )
        in_maps.append({"pq": pack_shard(shard), "cs": csb})
    return in_maps


def reduce_results(per_core_r, d_error):
    R = np.zeros((H, H), np.float64)
    for r in per_core_r:
        R += np.asarray(r).astype(np.float64)
    out = (np.asarray(d_error).astype(np.float64) * R).sum() / (B * E)
    return np.array(out, dtype=np.float32)


def run_spmd(P, circuit_edge_pairs, **kwargs):
    """Run on the 8 NeuronCores; returns (list of per-core R, BassKernelResults)."""
    from concourse.bass_utils import run_bass_kernel_spmd

    nc = get_nc()
    in_maps = make_in_maps(P, circuit_edge_pairs)
    res = run_bass_kernel_spmd(nc, in_maps, core_ids=list(range(N_CORES)), **kwargs)
    per_core_r = [res.results[c]["r_out"] for c in range(N_CORES)]
    return per_core_r, res


def kernel(P, d_error, circuit_edge_pairs, num_logical):
    assert int(num_logical) == L
    per_core_r, _ = run_spmd(P, circuit_edge_pairs)
    return reduce_results(per_core_r, d_error)
